# revision 15
# baseline (speedup 1.0000x reference)
"""Windowed correlation (cost volume) kernel for Trainium2, 8 NeuronCores.

Problem: feature1, feature2 (8, 128, 128, 256) fp32 -> out (8, 81, 128, 256),
out[b, ki*9+kj, y, x] = (1/128) * sum_c f1[b,c,y,x] * f2pad[b,c,y+ki,x+kj].

Strategy:
  - Data-parallel over batch: core i handles batch i (c=128 lands on the 128
    SBUF partitions; contraction over c runs on the TensorEngine).
  - Host marshals inputs: f1 is im2col-packed per (8y x 16x) pixel block and
    pre-cast to bf16; f2 is zero-padded (halo 4) fp32, cast to bf16 by the
    single SWDGE load DMA. Both live fully in SBUF.
  - Per pixel block, one bf16 matmul with lhsT = f1 block [c, 128pix] and
    rhs = the padded f2 halo block [c, 16*24=384] computes all pixel-pair
    products; the 81 useful products per pixel sit on diagonals. rhs blocks
    are im2col-staged per y0 row by one ACT copy (single-free-dim operands).
  - Diagonal extraction is impossible from SBUF/PSUM (partition-locked APs),
    so tiles round-trip through a DRAM scratch where a 3-dim strided
    (stride 385 = 384+1 "shear") DRAM->DRAM DMA gathers the 81 values per
    pixel as 9 contiguous kj-runs directly into a [y, x, 81] output layout.
  - Host transposes [b, y, x, d] -> [b, d, y, x] while unsharding.

Written in raw Bass (explicit blocks + semaphores): the walrus codegen
rejects instructions carrying more than one semaphore wait condition, so all
cross-engine waits are standalone wait_ge instructions managed by hand.

Engine plan per y0 row (pipelined by one iteration):
  ACT    f2row im2col copy (y0), then 36 shear-gathers of y0-1
  PE     16 matmuls (y0) into 4 rotating PSUM banks
  DVE    16 psum->stage copies with 1/128 scale (y0)
  SP     scratch store (y0), then 36 shear-gathers of y0-1
  GPSIMD single upfront f2p cast-load
"""

import numpy as np

_B, _C, _H, _W = 8, 128, 128, 256
_K = 9            # kernel size (2*max_disp+1)
_ND = _K * _K     # 81 displacements
_BY, _BX = 8, 16  # pixel block (M = _BY*_BX = 128 = PE rows)
_NBY, _NBX = _H // _BY, _W // _BX        # 16 x 16 blocks
_NA, _NB = _BY + _K - 1, _BX + _K - 1    # 16 x 24 halo block
_NCOLS = _NA * _NB                       # 384 psum columns
_HP, _WP = _H + _K - 1, _W + _K - 1      # padded f2 dims
_NPS = 4                                 # rotating psum banks

_CACHE = {}


def _gather_aps(bass, scr_t, out_t, y0):
    """Yield (src, dst) AP pairs for the shear-gathers of row y0.

    One DMA per (ki, x0): src walks [ry: part-row jump, rx: the 385 shear
    stride, kj: contiguous 9], dst is the matching [y, x, d] output block.
    """
    for ki in range(_K):
        for x0 in range(_NBX):
            src = bass.AP(
                tensor=scr_t,
                offset=y0 * _NBX * 128 * _NCOLS
                + x0 * 128 * _NCOLS
                + ki * _NB,
                ap=[[16 * _NCOLS + _NB, _BY], [_NCOLS + 1, _BX], [1, _K]],
            )
            dst = bass.AP(
                tensor=out_t,
                offset=(y0 * _BY * _W + x0 * _BX) * _ND + ki * _K,
                ap=[[_W * _ND, _BY], [_ND, _BX], [1, _K]],
            )
            yield src, dst


def _build_nc(reps: int = 1, gathers: bool = True):
    from contextlib import ExitStack

    import concourse.bass as bass
    import concourse.mybir as mybir

    nc = bass.Bass()
    # f1 comes in host-packed: [c, y0, x0*128 + ry*16 + rx] bf16
    f1 = nc.dram_tensor(
        "f1", [_C, _NBY, _NBX * 128], mybir.dt.bfloat16, kind="ExternalInput"
    )
    f2 = nc.dram_tensor("f2", [_C, _HP, _WP], mybir.dt.float32, kind="ExternalInput")
    out = nc.dram_tensor(
        "out", [_H, _W, _ND], mybir.dt.bfloat16, kind="ExternalOutput"
    )
    # whole-core scratch: one [16, 128, 384] slab per y0 row
    scr = nc.dram_tensor(
        "scr", [_NBY, _NBX, 128, _NCOLS], mybir.dt.bfloat16, kind="Internal"
    )

    inv_c = 1.0 / _C
    with ExitStack() as ctx:
        f1blk = ctx.enter_context(
            nc.sbuf_tensor([_C, _NBY * _NBX * 128], mybir.dt.bfloat16)
        )
        f2p = ctx.enter_context(nc.sbuf_tensor([_C, _HP * _WP], mybir.dt.bfloat16))
        f2row = [
            ctx.enter_context(
                nc.sbuf_tensor(f"f2r{i}", [_C, _NBX * _NCOLS], mybir.dt.bfloat16)
            )
            for i in range(2)
        ]
        stage = [
            ctx.enter_context(
                nc.sbuf_tensor(f"stg{i}", [_C, _NBX * _NCOLS], mybir.dt.bfloat16)
            )
            for i in range(2)
        ]
        psum = [
            ctx.enter_context(
                nc.psum_tensor(f"ps{i}", [128, _NCOLS], mybir.dt.float32)
            )
            for i in range(_NPS)
        ]
        s_f1 = ctx.enter_context(nc.semaphore(name="s_f1"))
        s_f2 = ctx.enter_context(nc.semaphore(name="s_f2"))
        s_act = ctx.enter_context(nc.semaphore(name="s_act"))   # +1 per f2row copy
        s_pe = ctx.enter_context(nc.semaphore(name="s_pe"))     # +1 per matmul
        s_dve = ctx.enter_context(nc.semaphore(name="s_dve"))   # +1 per stage copy
        s_st = ctx.enter_context(nc.semaphore(name="s_st"))     # +16 per scr store
        s_g = ctx.enter_context(nc.semaphore(name="s_g"))       # +16 per gather
        blk = ctx.enter_context(nc.Block())

        @blk.gpsimd
        def _(gpsimd):
            gpsimd.dma_start(f2p[:, :], f2.ap().rearrange("c h w -> c (h w)")).then_inc(
                s_f2, 16
            )

        rows = reps * _NBY
        per_row_g = _K * _NBX * 16  # s_g increments per fully-gathered row

        @blk.sync
        def _(sync):
            sync.dma_start(
                f1blk[:, :], f1.ap().rearrange("c a b -> c (a b)")
            ).then_inc(s_f1, 16)
            for r in range(rows):
                y0 = r % _NBY
                # store row r once its 16 stage copies are done
                sync.wait_ge(s_dve, (r + 1) * _NBX)
                if r >= _NBY and gathers:  # WAR: scr slab reused across reps
                    sync.wait_ge(s_g, (r - _NBY + 1) * per_row_g)
                scr_dst = bass.AP(
                    tensor=scr,
                    offset=y0 * _NBX * 128 * _NCOLS,
                    ap=[[_NCOLS, _C], [128 * _NCOLS, _NBX], [1, _NCOLS]],
                )
                sync.dma_start(scr_dst, stage[r % 2][:, :]).then_inc(s_st, 16)
                # first half of the shear-gathers for the previous row
                if r > 0 and gathers:
                    sync.wait_ge(s_st, r * 16)
                    for i, (src, dst) in enumerate(
                        _gather_aps(bass, scr, out, (r - 1) % _NBY)
                    ):
                        if i % 2 == 0:
                            sync.dma_start(dst, src).then_inc(s_g, 16)
            sync.wait_ge(s_st, rows * 16)
            if gathers:
                for i, (src, dst) in enumerate(
                    _gather_aps(bass, scr, out, (rows - 1) % _NBY)
                ):
                    if i % 2 == 0:
                        sync.dma_start(dst, src).then_inc(s_g, 16)
                # drain: all gathers complete
                sync.wait_ge(s_g, rows * per_row_g)

        @blk.scalar
        def _(scalar):
            scalar.wait_ge(s_f2, 16)
            for r in range(rows):
                y0 = r % _NBY
                # WAR: matmuls of r-2 read this f2row buffer
                if r >= 2:
                    scalar.wait_ge(s_pe, (r - 1) * _NBX)
                src2 = bass.AP(
                    tensor=f2p,
                    offset=y0 * _BY * _WP,
                    ap=[
                        [_HP * _WP, _C],
                        [_BX, _NBX],
                        [_WP, _NA],
                        [1, _NB],
                    ],
                )
                nc.scalar.activation(
                    f2row[r % 2][:, :], src2, mybir.ActivationFunctionType.Copy
                ).then_inc(s_act, 1)
                # second half of the shear-gathers for the previous row
                if r > 0 and gathers:
                    scalar.wait_ge(s_st, r * 16)
                    for i, (src, dst) in enumerate(
                        _gather_aps(bass, scr, out, (r - 1) % _NBY)
                    ):
                        if i % 2 == 1:
                            scalar.dma_start(dst, src).then_inc(s_g, 16)
            scalar.wait_ge(s_st, rows * 16)
            if gathers:
                for i, (src, dst) in enumerate(
                    _gather_aps(bass, scr, out, (rows - 1) % _NBY)
                ):
                    if i % 2 == 1:
                        scalar.dma_start(dst, src).then_inc(s_g, 16)

        @blk.tensor
        def _(tensor):
            tensor.wait_ge(s_f1, 16)
            for r in range(rows):
                y0 = r % _NBY
                tensor.wait_ge(s_act, r + 1)
                for x0 in range(_NBX):
                    n = r * _NBX + x0
                    if n >= _NPS:  # WAR: stage copy freed this psum bank
                        tensor.wait_ge(s_dve, n - _NPS + 1)
                    lhsT = f1blk[:, (y0 * _NBX + x0) * 128 : (y0 * _NBX + x0 + 1) * 128]
                    rhs = f2row[r % 2][:, x0 * _NCOLS : (x0 + 1) * _NCOLS]
                    nc.tensor.matmul(
                        psum[n % _NPS][:, :], lhsT, rhs, start=True, stop=True
                    ).then_inc(s_pe, 1)

        @blk.vector
        def _(vector):
            for r in range(rows):
                # WAR: store of r-2 read this stage buffer
                if r >= 2:
                    vector.wait_ge(s_st, (r - 1) * 16)
                for x0 in range(_NBX):
                    n = r * _NBX + x0
                    vector.wait_ge(s_pe, n + 1)
                    st = stage[r % 2][:, x0 * _NCOLS : (x0 + 1) * _NCOLS]
                    nc.vector.tensor_scalar_mul(
                        st, psum[n % _NPS][:, :], inv_c
                    ).then_inc(s_dve, 1)

    return nc


def _pack_f1(f1_core: np.ndarray) -> np.ndarray:
    """[c, h, w] fp32 -> [c, y0, x0*128 + ry*16 + rx] bf16."""
    import ml_dtypes

    v = f1_core.reshape(_C, _NBY, _BY, _NBX, _BX)
    v = v.transpose(0, 1, 3, 2, 4)  # c, y0, x0, ry, rx
    return np.ascontiguousarray(v.reshape(_C, _NBY, _NBX * 128)).astype(
        ml_dtypes.bfloat16
    )


def kernel(feature1: np.ndarray, feature2: np.ndarray) -> np.ndarray:
    from concourse.bass_utils import run_bass_kernel_spmd

    if "nc" not in _CACHE:
        _CACHE["nc"] = _build_nc()
    nc = _CACHE["nc"]

    f1 = np.ascontiguousarray(np.asarray(feature1), dtype=np.float32)
    f2 = np.ascontiguousarray(np.asarray(feature2), dtype=np.float32)
    f2p = np.zeros((_B, _C, _HP, _WP), dtype=np.float32)
    f2p[:, :, 4 : 4 + _H, 4 : 4 + _W] = f2
    in_maps = [{"f1": _pack_f1(f1[i]), "f2": f2p[i]} for i in range(_B)]
    res = run_bass_kernel_spmd(nc, in_maps, core_ids=list(range(_B)))
    # [b, y, x, d] bf16 -> [b, d, y, x] fp32
    out = np.stack([res.results[i]["out"] for i in range(_B)], axis=0)
    return np.ascontiguousarray(out.transpose(0, 3, 1, 2)).astype(np.float32)
